# revision 93
# baseline (speedup 1.0000x reference)
"""AttentionBlock3D (GroupNorm + 8-head attention + proj + residual) on 8 trn2 cores.

Sharding: core i handles (batch b = i//4, query-quarter qs = i%4).
Each core redundantly computes full K/V for its batch (cheap) and exclusively
computes Q/attention/projection for its 1024 spatial positions. No inter-core
communication; the host concatenates the 8 output slices.

v2 design (engine-balanced; softmax exps split across ACT and DVE):
  - GroupNorm folded into the QKV weights ON DEVICE: after bn_stats, weight
    rows are scaled by rstd (per input channel) and biases get +W@(-mu*rstd),
    so Q/K/V matmuls read RAW x straight from the DMA (no normalize pass).
  - The V bias is folded further into the projection bias (softmax rows sum
    to 1), and the proj bias is pre-added into the residual source in-place.
  - QK^T in f32r (full PE rate at 512 moving), two heads per PSUM tile via
    tile_position 32-row K strips. Softmax skips max-subtraction.
  - exp: ACT computes bf16 exp from PSUM; DVE computes Schraudolph bf16 exp
    (int16 = x*scale*128*log2e + bias, bit-pattern == bf16) via a bitcast
    view. Engine chosen per step by a tunable pattern; both stream in
    parallel, roughly halving the softmax time vs ACT-only.
  - AV transposed + bf16: out[q,dh] = sum_k ex[k,q]*V[k,dh] with ex as the
    128-wide stationary operand and V (32 cols + ones col for the softmax
    denominator) moving: 33 PE cycles per (kb, head, q-block) instead of 512.
  - O normalize is then a per-partition scalar multiply (reciprocal of the
    denominator column), output in bf16; per chunk O is transposed back to
    channel-major by 8 PE transposes and projected with bf16 weights. The
    residual (+ pre-folded bias) is added on the PE via an identity matmul.
  - AV matmuls lag the QK/exp stream by up to MAXLAG steps (drained at each
    wave end) so the exp pipeline never stalls on the in-order PE queue or
    the single-buffered AV accumulator bank.
  - PSUM: 3x2-bank score ring + 1 AV bank + 1 work bank = 8 banks. Multiple
    AV accumulation series share one bank, so only the first matmul uses
    start=True (start marks the whole 2KB zero-region pending-zero).
"""

import numpy as np

B, C, N = 2, 256, 4096
HEADS, GROUPS = 8, 8
DH = C // HEADS  # 32
NQ = N // 4      # queries per core
EPS = 1e-5
N_CORES = 8
NKB = N // 128   # 32 key blocks
SCALE = 1.0 / float(np.sqrt(DH))
LOG2E = float(np.log2(np.e))
# Schraudolph bf16 exp: i16 = trunc(s*SCALE*128*log2e + (16256 + 0.5 - C_ADJ))
C_ADJ = 5.5
SCH_A = SCALE * 128.0 * LOG2E
SCH_B = 16256.0 + 0.5 - C_ADJ

# exp engine pattern per wave (32 chars each, 'A'=ACT, 'D'=DVE).
# wave 0 carries V-evac on ACT, wave 2 carries K/Q-evac on ACT -> DVE-heavy.
def _pat(n_act, phase=0):
    s = []
    acc = 0
    for i in range(NKB):
        acc += n_act
        if acc >= NKB:
            acc -= NKB
            s.append('A')
        else:
            s.append('D')
    return ''.join(s)

EXP_PAT = [
    _pat(13),  # wave 0 (V emission, evacs on ACT)
    _pat(18),
    _pat(14),  # wave 2 (K/Q j=1 emission evacs on ACT)
    _pat(18),
    _pat(18),
    _pat(18),
    _pat(18),
    _pat(18),
]

LAST_RESULTS = None  # BassKernelResults of the most recent run (for test.py)


def _build_program():
    import concourse.bass as bass
    import concourse.bacc as bacc
    import concourse.tile as tile
    from concourse import mybir

    f32 = mybir.dt.float32
    f32r = mybir.dt.float32r
    bf16 = mybir.dt.bfloat16
    i16 = mybir.dt.int16
    Alu = mybir.AluOpType
    Act = mybir.ActivationFunctionType

    nc = bacc.Bacc("TRN2", target_bir_lowering=False)

    # ---- DRAM I/O ----
    x_d = nc.dram_tensor("x", [C, N], f32r, kind="ExternalInput")
    wqT_d = nc.dram_tensor("wqT", [C, C], f32, kind="ExternalInput")
    wkT_d = nc.dram_tensor("wkT", [C, C], f32, kind="ExternalInput")
    wvT_d = nc.dram_tensor("wvT", [C, C], f32, kind="ExternalInput")
    wpT_d = nc.dram_tensor("wpT", [C, C], f32, kind="ExternalInput")
    bq_d = nc.dram_tensor("bq", [C, 1], f32, kind="ExternalInput")
    bk_d = nc.dram_tensor("bk", [C, 1], f32, kind="ExternalInput")
    bv_d = nc.dram_tensor("bv", [C, 1], f32, kind="ExternalInput")
    bp_d = nc.dram_tensor("bp", [C, 1], f32, kind="ExternalInput")
    gmap_d = nc.dram_tensor("gmap", [2, 128, GROUPS], f32, kind="ExternalInput")
    bmap_d = nc.dram_tensor("bmap", [2, GROUPS, 128], f32, kind="ExternalInput")
    id_d = nc.dram_tensor("ident", [128, 128], f32, kind="ExternalInput")
    out_d = nc.dram_tensor("out", [C, NQ], f32, kind="ExternalOutput")
    # the host passes x pre-rolled so the query quarter is always cols 0:NQ

    with tile.TileContext(nc) as tc:
        with (
            tc.tile_pool(name="const", bufs=1) as const,
            tc.tile_pool(name="data", bufs=1) as data,
            tc.tile_pool(name="tmp", bufs=2) as tmp,
            tc.tile_pool(name="exps", bufs=14) as exps,
            tc.tile_pool(name="psA", bufs=3, space="PSUM") as psA,
            tc.tile_pool(name="psV", bufs=1, space="PSUM") as psV,
            tc.tile_pool(name="psW", bufs=1, space="PSUM") as psW,
        ):
            # ---- tiny group-map consts first (HWDGE, ahead of the x bulk) ----
            gmap_sb = [const.tile([128, GROUPS], f32, name=f"gmap{j}") for j in range(2)]
            bmap_sb = [const.tile([GROUPS, 128], f32, name=f"bmap{j}") for j in range(2)]
            # ---- x load FIRST: 4 col-chunks per half, two HWDGE queues ----
            xt = [data.tile([128, N], f32r, name=f"xt{j}") for j in range(2)]
            xsl = [slice(0, 1024), slice(1024, 2048), slice(2048, 3072),
                   slice(3072, 3584), slice(3584, 4096)]
            for csl in xsl:
                nc.sync.dma_start(out=xt[0][:, csl], in_=x_d[0:128, csl])
                nc.scalar.dma_start(out=xt[1][:, csl], in_=x_d[128:256, csl])
            gstg = [tmp.tile([128, GROUPS], f32, tag="gstg", name=f"gstg{j}", bufs=2)
                    for j in range(2)]
            bstg = [tmp.tile([GROUPS, 128], f32, tag="bstg", name=f"bstg{j}", bufs=2)
                    for j in range(2)]
            for j in range(2):
                nc.gpsimd.dma_start(out=gstg[j], in_=gmap_d[j])
                nc.gpsimd.dma_start(out=bstg[j], in_=bmap_d[j])
            # (DVE copies into gmap_sb/bmap_sb are deferred until after the
            # bn_stats emission so they don't block the stats pipeline)
            bhost = {}
            for nm, d in (("bq", bq_d), ("bk", bk_d), ("bv", bv_d), ("bp", bp_d)):
                bhost[nm] = [const.tile([128, 1], f32, name=f"{nm}{j}") for j in range(2)]
                for j in range(2):
                    nc.gpsimd.dma_start(out=bhost[nm][j], in_=d[j * 128:(j + 1) * 128, :])
            id_stg = const.tile([128, 128], f32, name="id_stg")
            nc.gpsimd.dma_start(out=id_stg, in_=id_d[:, :])
            id_bf = const.tile([128, 128], bf16, name="id_bf")
            id_r = const.tile([128, 128], f32r, name="id_r")
            # (identity copies are emitted after the stats chain; emitting
            # them here would block the in-order DVE queue on the SWDGE DMA)

            # ACT table prewarm: pulls the ln/exp table load off the critical path
            warm = tmp.tile([8, 1], f32, tag="warm", bufs=1)
            nc.vector.memset(warm, 0.0)
            nc.scalar.activation(out=warm, in_=warm, func=Act.Exp)

            # PE pstate prewarm: ~3.5us of dummy matmuls during the DMA phase
            # ramp the tensor engine to full clock before the real chain
            wmm = tmp.tile([128, 512], f32, tag="wmm", bufs=1)
            nc.vector.memset(wmm, 0.0)
            wps = psW.tile([128, 512], f32, tag="work", name="wps")
            for i in range(9):
                nc.tensor.matmul(wps, wmm[:, 0:128], wmm, start=True, stop=True)

            # ---- weights staging (f32, HWDGE queues after x) ----
            wstg = {}
            for wi, wd in enumerate((wqT_d, wkT_d, wvT_d, wpT_d)):
                for kk in range(2):
                    t = const.tile([128, C], f32, name=f"wstg{wi}{kk}")
                    eng = nc.sync if kk == 0 else nc.scalar
                    eng.dma_start(out=t, in_=wd[kk * 128:(kk + 1) * 128, :])
                    wstg[(wi, kk)] = t

            # ---- GroupNorm statistics via bn_stats (one DVE pass over x),
            # interleaved in chunk-arrival order so DVE starts immediately ----
            st = [tmp.tile([128, 2], f32, name=f"st{j}", tag="st", bufs=2) for j in range(2)]
            bnst = [tmp.tile([128, 8, 6], f32, tag=f"bnst{j}", bufs=1, name=f"bnst{j}")
                    for j in range(2)]
            mv = [tmp.tile([128, 2], f32, tag=f"mv{j}", bufs=1, name=f"mv{j}")
                  for j in range(2)]
            for ch in range(4):
                for j in range(2):
                    for sub in (2 * ch, 2 * ch + 1):
                        nc.vector.bn_stats(
                            out=bnst[j][:, sub, :],
                            in_=xt[j][:, sub * 512:(sub + 1) * 512],
                        )
                    if ch == 3:
                        nc.vector.bn_aggr(out=mv[j], in_=bnst[j])
                        nc.vector.tensor_copy(out=st[j][:, 0:1], in_=mv[j][:, 0:1])
                        nc.vector.tensor_mul(
                            out=st[j][:, 1:2], in0=mv[j][:, 0:1], in1=mv[j][:, 0:1])
                        nc.vector.tensor_add(
                            out=st[j][:, 1:2], in0=st[j][:, 1:2], in1=mv[j][:, 1:2])
            for j in range(2):
                nc.vector.tensor_copy(out=gmap_sb[j], in_=gstg[j])
                nc.vector.tensor_copy(out=bmap_sb[j], in_=bstg[j])
            stats_ps = psW.tile([GROUPS, 2], f32, tag="work", name="stats_ps")
            for j in range(2):
                nc.tensor.matmul(
                    stats_ps, gmap_sb[j], st[j], start=(j == 0), stop=(j == 1),
                )
            inv_n = 1.0 / (C // GROUPS)
            ms = tmp.tile([GROUPS, 2], f32, tag="ms", bufs=1)  # [mu | rstd]
            nc.vector.tensor_scalar_mul(out=ms[:, 0:1], in0=stats_ps[:, 0:1], scalar1=inv_n)
            ve = tmp.tile([GROUPS, 1], f32, tag="ve", bufs=1)
            nc.vector.tensor_scalar_mul(out=ve, in0=stats_ps[:, 1:2], scalar1=inv_n)
            musq = tmp.tile([GROUPS, 1], f32, tag="musq", bufs=1)
            nc.vector.tensor_mul(out=musq, in0=ms[:, 0:1], in1=ms[:, 0:1])
            nc.vector.tensor_sub(out=ve, in0=ve, in1=musq)
            nc.vector.tensor_scalar_add(out=ve, in0=ve, scalar1=EPS)
            # rsqrt fully on DVE (quake seed + 3 Newton steps): keeps the
            # ACT table on the exp set (no Ln/Exp table swaps on the
            # startup critical path)
            i32 = mybir.dt.int32
            magic = tmp.tile([GROUPS, 1], i32, tag="magic", bufs=1)
            nc.vector.memset(magic, 0x5F3759DF)
            r0 = tmp.tile([GROUPS, 1], f32, tag="r0", bufs=1)
            half_i = tmp.tile([GROUPS, 1], i32, tag="half_i", bufs=1)
            nc.vector.tensor_scalar(
                out=half_i, in0=ve[:, 0:1].bitcast(i32), scalar1=1,
                scalar2=None, op0=Alu.logical_shift_right,
            )
            nc.vector.tensor_sub(
                out=r0[:, 0:1].bitcast(i32), in0=magic, in1=half_i,
            )
            t_nw = tmp.tile([GROUPS, 1], f32, tag="t_nw", bufs=1)
            for _ in range(1):
                nc.vector.tensor_mul(out=t_nw, in0=r0, in1=r0)
                nc.vector.tensor_mul(out=t_nw, in0=t_nw, in1=ve)
                nc.vector.tensor_scalar(
                    out=t_nw, in0=t_nw, scalar1=-0.5, scalar2=1.5,
                    op0=Alu.mult, op1=Alu.add,
                )
                nc.vector.tensor_mul(out=r0, in0=r0, in1=t_nw)
            nc.vector.tensor_copy(out=ms[:, 1:2], in_=r0)

            # broadcast (mu, rstd) to per-partition columns; c = -mu*rstd as
            # f32 rhs for the bias matvecs (fp32r disallows 1-wide moving)
            musc = []
            cvec = []
            for j in range(2):
                bc_ps = psW.tile([128, 2], f32, tag="work", name=f"bc_ps{j}")
                nc.tensor.matmul(bc_ps, bmap_sb[j], ms, start=True, stop=True)
                m = tmp.tile([128, 2], f32, tag="musc", bufs=2, name=f"musc{j}")
                nc.vector.tensor_copy(out=m, in_=bc_ps)
                musc.append(m)
                cv = tmp.tile([128, 1], f32, tag="cvec", bufs=2, name=f"cvec{j}")
                nc.vector.tensor_scalar(
                    out=cv, in0=m[:, 0:1], scalar1=m[:, 1:2], scalar2=-1.0,
                    op0=Alu.mult, op1=Alu.mult,
                )
                cvec.append(cv)

            # ---- fold GN into weights: w_eff = w * rstd (per input channel);
            # scaled lazily per weight so K's emission overlaps Q/V scaling ----
            w_eff = {}

            def scale_w(wi):
                for kk in range(2):
                    t = const.tile([128, C], f32r, name=f"weff{wi}{kk}")
                    nc.vector.tensor_scalar_mul(
                        out=t, in0=wstg[(wi, kk)], scalar1=musc[kk][:, 1:2],
                    )
                    w_eff[(wi, kk)] = t
            wp_bf = []  # filled by emit_bp_chain (deferred into wave 0)

            # ---- effective biases: b_eff = b_host + W_eff @ (-mu) ----
            def bias_matvec(wi, j, lhs_tiles, rhs_tiles):
                ps = psW.tile([128, 1], f32, tag="work", name=f"bps{wi}{j}")
                for kk in range(2):
                    nc.tensor.matmul(
                        ps, lhs_tiles[kk][:, j * 128:(j + 1) * 128], rhs_tiles[kk],
                        start=(kk == 0), stop=(kk == 1),
                    )
                return ps

            b_eff = {}

            def emit_beff(wi, nm):
                # b_eff = b_host + W_f @ (-mu*rstd): f32 staged weights
                b_eff[nm] = []
                for j in range(2):
                    ps = bias_matvec(wi, j, [wstg[(wi, 0)], wstg[(wi, 1)]], cvec)
                    t = tmp.tile([128, 1], f32, tag=f"beff{nm}", bufs=2, name=f"beff{nm}{j}")
                    nc.vector.tensor_add(out=t, in0=bhost[nm][j], in1=ps)
                    b_eff[nm].append(t)

            # bq/bk needed by the first evacs; bv/bp deferred into wave 1
            scale_w(1)
            emit_beff(1, "bk")

            def emit_bp_chain():
                for kk in range(2):
                    t = const.tile([128, C], bf16, name=f"wpbf{kk}")
                    nc.vector.tensor_copy(out=t, in_=wstg[(3, kk)])
                    wp_bf.append(t)
                emit_beff(2, "bv")
                bv_bf = []
                for j in range(2):
                    t = tmp.tile([128, 1], bf16, tag="bvbf", bufs=2, name=f"bvbf{j}")
                    nc.vector.tensor_copy(out=t, in_=b_eff["bv"][j])
                    bv_bf.append(t)
                # bp_eff = bp + Wp @ bv_eff (pre-added into the residual later)
                for j in range(2):
                    ps = bias_matvec(3, j, wp_bf, bv_bf)
                    t = tmp.tile([128, 1], f32, tag="beffbp", bufs=2, name=f"beffbp{j}")
                    nc.vector.tensor_add(out=t, in0=bhost["bp"][j], in1=ps)
                    b_eff.setdefault("bp", []).append(t)

            # ---- K/Q/V emission from raw x ----
            K_sb = [data.tile([128, N], f32r, name=f"K{j}") for j in range(2)]
            Q_sb = [data.tile([128, NQ], f32r, name=f"Q{j}") for j in range(2)]

            def emit_q(j, n):
                ps = psW.tile([128, 512], f32, tag="work", name="qps")
                for kk in range(2):
                    nc.tensor.matmul(
                        ps,
                        w_eff[(0, kk)][:, j * 128:(j + 1) * 128],
                        xt[kk][:, n * 512:(n + 1) * 512],
                        start=(kk == 0), stop=(kk == 1),
                    )
                nc.scalar.activation(
                    out=Q_sb[j][:, n * 512:(n + 1) * 512], in_=ps,
                    func=Act.Identity, bias=b_eff["bq"][j],
                )

            def emit_k(j, n):
                ps = psW.tile([128, 512], f32, tag="work", name="kps")
                for kk in range(2):
                    nc.tensor.matmul(
                        ps,
                        w_eff[(1, kk)][:, j * 128:(j + 1) * 128],
                        xt[kk][:, n * 512:(n + 1) * 512],
                        start=(kk == 0), stop=(kk == 1),
                    )
                nc.scalar.activation(
                    out=K_sb[j][:, n * 512:(n + 1) * 512], in_=ps,
                    func=Act.Identity, bias=b_eff["bk"][j],
                )

            # V^T [128, kb, 8, 33] bf16: 32 value cols + ones col per head
            V_sb = data.tile([128, NKB, HEADS, DH + 1], bf16)
            vones = const.tile([128, NKB * HEADS], bf16)
            nc.vector.memset(vones, 1.0)
            nc.vector.tensor_copy(
                out=V_sb[:, :, :, DH:DH + 1],
                in_=vones.rearrange("p (kb h o) -> p kb h o", h=HEADS, o=1),
            )

            def emit_v(kb2):
                # two key blocks per PSUM round: halves the work-ring round
                # trips (work pool is single-buffered)
                ps = psW.tile([128, 512], f32, tag="work", name="vps")
                for sub in range(2):
                    for kk in range(2):
                        nc.tensor.matmul(
                            ps[:, sub * C:(sub + 1) * C],
                            xt[kk][:, (2 * kb2 + sub) * 128:(2 * kb2 + sub + 1) * 128],
                            w_eff[(2, kk)],
                            start=(kk == 0), stop=(kk == 1),
                        )
                src = ps.rearrange("p (s h x) -> p s h x", s=2, h=HEADS)
                dst = V_sb[:, 2 * kb2:2 * kb2 + 2, :, 0:DH]
                nc.scalar.activation(out=dst, in_=src, func=Act.Identity)

            # pre-wave K/Q j=0 emission through the (still idle) psA ring:
            # [128, 2, 512] PSUM rounds, double-width ACT evacs, 3-deep
            def emit_kq_pre(wi, dst, bias, j, n2, eng):
                ps = psA.tile([128, 2, 512], f32, tag="scores", name="kqps")
                for nb in range(2):
                    n = 2 * n2 + nb
                    for kk in range(2):
                        nc.tensor.matmul(
                            ps[:, nb, :],
                            w_eff[(wi, kk)][:, j * 128:(j + 1) * 128],
                            xt[kk][:, n * 512:(n + 1) * 512],
                            start=(kk == 0), stop=(kk == 1),
                        )
                dsl = dst[j][:, 2 * n2 * 512:(2 * n2 + 2) * 512]
                if eng == 'A':
                    nc.scalar.activation(
                        out=dsl, in_=ps.rearrange("p a b -> p (a b)"),
                        func=Act.Identity, bias=bias[j],
                    )
                else:
                    nc.vector.tensor_scalar_add(
                        out=dsl, in0=ps.rearrange("p a b -> p (a b)"),
                        scalar1=bias[j],
                    )

            emit_kq_pre(1, K_sb, b_eff["bk"], 0, 0, 'A')
            scale_w(0)
            emit_beff(0, "bq")
            emit_kq_pre(0, Q_sb, b_eff["bq"], 0, 0, 'D')
            scale_w(2)
            for n2 in range(1, 4):
                emit_kq_pre(1, K_sb, b_eff["bk"], 0, n2, 'D')
            nc.vector.tensor_copy(out=id_bf, in_=id_stg)
            nc.vector.tensor_copy(out=id_r, in_=id_stg)

            # ---- attention ----
            Oq = [data.tile([128, 4, C], bf16, name=f"Oq{c}") for c in range(2)]
            OT_sb = [data.tile([128, 2, 512], bf16, name=f"OT{c}") for c in range(2)]
            out_sb = [data.tile([128, NQ], f32, name=f"outsb{j}") for j in range(2)]

            def make_step(info, kb, ex):
                def emit():
                    if info["av"] is None:
                        info["av"] = psV.tile(
                            [128, 4, 2, DH + 1], f32, tag="avot", name="av",
                        )
                    av = info["av"]
                    last = (kb == NKB - 1)
                    for qsub in range(4):
                        for hx in range(2):
                            # start only the FIRST series: start_tensor_calc
                            # marks the whole 2KB zero-region pending-zero, so
                            # later first-touches overwrite (= implicit start)
                            first = (kb == 0) and (qsub == 0) and (hx == 0)
                            nc.tensor.matmul(
                                av[:, qsub, hx, :],
                                ex[:, hx, qsub * 128:(qsub + 1) * 128],
                                V_sb[:, kb, info["hA"] + hx, :],
                                start=first, stop=last, skip_group_check=True,
                                tile_position=(0, 0),
                            )
                    if last:
                        # normalize: O = AV * (1/denom); denom is col DH.
                        # both heads' output columns are adjacent, so one
                        # 4-dim AP covers the whole multiply
                        rc = tmp.tile([128, 4, 2], f32, tag="rc", name="rc", bufs=2)
                        nc.vector.reciprocal(out=rc, in_=av[:, :, :, DH])
                        c = info["c"]
                        hA = info["hA"]
                        nc.vector.tensor_tensor(
                            out=Oq[c][:, :, hA * DH:(hA + 2) * DH].rearrange(
                                "p a (hx x) -> p a hx x", hx=2),
                            in0=av[:, :, :, 0:DH],
                            in1=rc[:, :, :].to_broadcast([128, 4, 2, DH]),
                            op=Alu.mult,
                        )
                return emit

            def emit_ot(c, j):
                # transpose half j of chunk c back to channel-major
                ot = psW.tile([128, 4, 128], bf16, tag="work", name=f"ot{j}")
                for qsub in range(4):
                    nc.tensor.transpose(
                        ot[:, qsub, :],
                        Oq[c][:, qsub, j * 128:(j + 1) * 128],
                        id_bf,
                    )
                nc.scalar.activation(
                    out=OT_sb[c][:, j, :],
                    in_=ot.rearrange("p a b -> p (a b)"),
                    func=Act.Identity,
                )

            MAXLAG = 10
            wave_i = 0
            pending = []
            for c in range(NQ // 512):
                qsl = slice(c * 512, (c + 1) * 512)
                for p in range(4):
                    hA, hB = 2 * p, 2 * p + 1
                    jt = hA // 4
                    sA, sB = 32 * (hA % 4), 32 * (hB % 4)
                    info = {"hA": hA, "c": c, "av": None}
                    pat = EXP_PAT[wave_i]
                    for kb in range(NKB):
                        if wave_i == 0:
                            if kb % 2 == 0:
                                emit_v(kb // 2)
                        elif wave_i == 1:
                            if kb == 8:
                                emit_bp_chain()
                        elif wave_i == 2:
                            # K/Q j=1 just-in-time for this wave's QKs
                            if kb < 2:
                                emit_q(1, kb)
                            if kb % 4 == 0:
                                emit_k(1, kb // 4)
                        if p == 3 and kb == 2:
                            # heads 0-3 of this chunk are normalized; start
                            # the j=0 transposes early
                            emit_ot(c, 0)
                        sc = psA.tile([128, 2, 512], f32, tag="scores", name="sc")
                        ksl = slice(kb * 128, (kb + 1) * 128)
                        nc.tensor.matmul(
                            sc[:, 0, :],
                            K_sb[jt][sA:sA + 32, ksl],
                            Q_sb[jt][sA:sA + 32, qsl],
                            start=True, stop=True, tile_position=(sA, 0),
                        )
                        nc.tensor.matmul(
                            sc[:, 1, :],
                            K_sb[jt][sB:sB + 32, ksl],
                            Q_sb[jt][sB:sB + 32, qsl],
                            start=True, stop=True, tile_position=(sB, 0),
                        )
                        ex = exps.tile([128, 2, 512], bf16, tag="ex", name="ex")
                        if pat[kb] == 'A':
                            nc.scalar.activation(
                                out=ex, in_=sc, func=Act.Exp, scale=SCALE,
                            )
                        else:
                            nc.vector.tensor_scalar(
                                out=ex[:, :, :].bitcast(i16), in0=sc,
                                scalar1=SCH_A, scalar2=SCH_B,
                                op0=Alu.mult, op1=Alu.add,
                            )
                        pending.append(make_step(info, kb, ex))
                        while len(pending) > MAXLAG:
                            pending.pop(0)()
                    if wave_i == 2:
                        # all xt reads done; fold proj bias into residual cols
                        # (gpsimd: SBUF-only op, keeps DVE free mid-stream)
                        for j in range(2):
                            nc.gpsimd.tensor_scalar_add(
                                out=xt[j][:, 0:NQ], in0=xt[j][:, 0:NQ],
                                scalar1=b_eff["bp"][j],
                            )
                    wave_i += 1
                    # drain the lag at each wave end so the normalize starts
                    # immediately and the next wave's AV has slack on the
                    # single-buffered av bank
                    while pending:
                        pending.pop(0)()

                # ---- transpose half j=1 (j=0 was emitted early) ----
                emit_ot(c, 1)

                # ---- proj + residual(+bias) for this query chunk ----
                for j in range(2):
                    # j=1 uses the av bank (free at chunk end) so the two
                    # projections don't serialize on the work ring
                    pool_ = psW if j == 0 else psV
                    tag_ = "work" if j == 0 else "avot"
                    pp = pool_.tile([128, 512], f32, tag=tag_, name="pps")
                    for kk in range(2):
                        nc.tensor.matmul(
                            pp,
                            wp_bf[kk][:, j * 128:(j + 1) * 128],
                            OT_sb[c][:, kk, :],
                            start=(kk == 0), stop=False,
                        )
                    # residual (+ pre-folded proj bias) added on the PE via
                    # the identity: pp += I^T @ x
                    nc.tensor.matmul(
                        pp, id_r, xt[j][:, qsl], start=False, stop=True,
                    )
                    nc.scalar.activation(
                        out=out_sb[j][:, qsl], in_=pp, func=Act.Identity,
                    )
                    eng_dma = nc.sync if j == 0 else nc.scalar
                    eng_dma.dma_start(
                        out=out_d[j * 128:(j + 1) * 128, qsl],
                        in_=out_sb[j][:, qsl],
                    )

    nc.compile()
    return nc


_NC_CACHE = None


def kernel(x, gamma, beta, w_qkv, b_qkv, w_proj, b_proj):
    global LAST_RESULTS, _NC_CACHE
    from concourse.bass_utils import run_bass_kernel_spmd

    x = np.ascontiguousarray(np.asarray(x, np.float32))
    gamma = np.asarray(gamma, np.float32)
    beta = np.asarray(beta, np.float32)
    w_qkv = np.asarray(w_qkv, np.float32)
    b_qkv = np.asarray(b_qkv, np.float32)
    w_proj = np.asarray(w_proj, np.float32)
    b_proj = np.asarray(b_proj, np.float32)

    # Fold GroupNorm's gamma/beta into the QKV conv (per-voxel linear):
    #   qkv(hn*g + b) = (w*g) @ hn + (b_qkv + w @ b)
    w_f = w_qkv * gamma[None, :]
    b_f = b_qkv + w_qkv @ beta
    wqT = np.ascontiguousarray(w_f[0:C].T)
    wkT = np.ascontiguousarray(w_f[C:2 * C].T)
    wvT = np.ascontiguousarray(w_f[2 * C:3 * C].T)
    wpT = np.ascontiguousarray(w_proj.T)
    bq = np.ascontiguousarray(b_f[0:C].reshape(C, 1))
    bk = np.ascontiguousarray(b_f[C:2 * C].reshape(C, 1))
    bv = np.ascontiguousarray(b_f[2 * C:3 * C].reshape(C, 1))
    bp = np.ascontiguousarray(b_proj.reshape(C, 1))

    part = np.arange(128)
    gmap = np.zeros((2, 128, GROUPS), np.float32)
    bmap = np.zeros((2, GROUPS, 128), np.float32)
    for j in range(2):
        g_of_p = (part + 128 * j) // (C // GROUPS)
        gmap[j, part, g_of_p] = 1.0
        bmap[j, g_of_p, part] = 1.0
    ident = np.eye(128, dtype=np.float32)

    xf = x.reshape(B, C, N)
    in_maps = []
    for core in range(N_CORES):
        b, qs = core // 4, core % 4
        # roll so this core's query quarter occupies columns 0:NQ
        xr = np.roll(xf[b], -qs * NQ, axis=1)
        in_maps.append({
            "x": np.ascontiguousarray(xr),
            "wqT": wqT, "wkT": wkT, "wvT": wvT, "wpT": wpT,
            "bq": bq, "bk": bk, "bv": bv, "bp": bp,
            "gmap": gmap, "bmap": bmap, "ident": ident,
        })

    if _NC_CACHE is None:
        _NC_CACHE = _build_program()
    res = run_bass_kernel_spmd(_NC_CACHE, in_maps, list(range(N_CORES)))
    LAST_RESULTS = res

    out = np.empty((B, C, N), np.float32)
    for core in range(N_CORES):
        b, qs = core // 4, core % 4
        out[b][:, qs * NQ:(qs + 1) * NQ] = res.results[core]["out"]
    return out.reshape(B, C, 16, 16, 16)
